# revision 11
# baseline (speedup 1.0000x reference)
"""Trainium2 Bass kernel for causal multi-head attention (B=2, T=2048, C=1024, H=16).

Reference semantics:
    qp = q @ Wq + bq ; kp = k @ Wk + bk ; vp = kp @ Wv + bv   (V from projected K)
    S  = (qh @ khT) / sqrt(D), causal mask, A = softmax(S)
    ctx = A @ vh ; out = ctx @ Wo + bo
Returns (out, attention_weights).

Sharding: 8 cores; core c handles batch b = c//4 and 4 heads h0 = 4*(c%4).
Each core gets transposed activations qT/kT [C, T], head-sliced weights, and
host-folded Wkv = Wk @ Wv[:, slice] so vp comes straight from k. The scores
scale 1/sqrt(D) is folded into Wq/bq. Partial output projections are summed on
the host (the all-reduce of the sharding strategy); bo and the upper-triangle
zeros of attention_weights are applied on the host as well.

Device data flow per core (all matmul operands float32r ~ 12-bit mantissa):
  qpT/kpT [256, T] (features on partitions), vp65 [T, 4, 65] (per-head v
  columns + a ones column that makes the PV matmul also produce the softmax
  denominator row).
  Per head pair (partition offsets 0/64 -> concurrent PE row groups) and
  q-window of 512:
    A-pass: S = qh^T-tile @ kh window (K=64 matmuls, paired), additive causal
    mask on the diagonal block, exp on ScalarE with fused row-sum accum,
    normalize by reciprocal row sum, DMA the causal part of A out.
    ST-pass: S^T tiles [k-tile, q-window] straight from kpT/qpT (paired),
    exp -> E^T (unnormalized), PV matmul ctxT[65, 512] += vp65^T @ E^T whose
    row 64 is the denominator; ctxT rows 0..63 are multiplied by the
    broadcast reciprocal denominator (K=1 ones outer product on the PE).
  Output projection from ctxT with the Wo slice.
"""
import math
import numpy as np
from contextlib import ExitStack

import concourse.mybir as mybir
import concourse.tile as tile
from concourse import bacc
from concourse.bass_utils import run_bass_kernel_spmd
from concourse.masks import make_causal_mask

B, T, C, H = 2, 2048, 1024, 16
D = C // H            # 64
NCORES = 8
HPC = 4               # heads per core
HD = HPC * D          # 256
CK = C // 128         # 8 contraction chunks
QT = T // 128         # 16 q tiles
F32 = mybir.dt.float32
RDT = mybir.dt.float32r   # matmul operand dtype

ACT = mybir.ActivationFunctionType


def _build():
    nc = bacc.Bacc("TRN2", target_bir_lowering=False, debug=False,
                   num_devices=NCORES)
    qT = nc.dram_tensor("qT", [C, T], F32, kind="ExternalInput").ap()
    kT = nc.dram_tensor("kT", [C, T], F32, kind="ExternalInput").ap()
    wq = nc.dram_tensor("wq", [C, HD], F32, kind="ExternalInput").ap()
    wk = nc.dram_tensor("wk", [C, HD], F32, kind="ExternalInput").ap()
    wkv = nc.dram_tensor("wkv", [C, HD], F32, kind="ExternalInput").ap()
    wo = nc.dram_tensor("wo", [HD, C], F32, kind="ExternalInput").ap()
    bq = nc.dram_tensor("bq", [2, 128, 1], F32, kind="ExternalInput").ap()
    bk = nc.dram_tensor("bk", [2, 128, 1], F32, kind="ExternalInput").ap()
    bkv = nc.dram_tensor("bkv", [1, HD], F32, kind="ExternalInput").ap()
    attnw = nc.dram_tensor("attnw", [HPC, T, T], F32, kind="ExternalOutput").ap()
    outp = nc.dram_tensor("outp", [T, C], F32, kind="ExternalOutput").ap()

    with tile.TileContext(nc) as tc, ExitStack() as ctx:
        consts = ctx.enter_context(tc.tile_pool(name="consts", bufs=1))
        wpool = ctx.enter_context(tc.tile_pool(name="wpool", bufs=1))
        big = ctx.enter_context(tc.tile_pool(name="big", bufs=1))
        acts = ctx.enter_context(tc.tile_pool(name="acts", bufs=1))
        apool = ctx.enter_context(tc.tile_pool(name="apool", bufs=4))
        etp = ctx.enter_context(tc.tile_pool(name="etp", bufs=4))
        small = ctx.enter_context(tc.tile_pool(name="small", bufs=4))
        opool = ctx.enter_context(tc.tile_pool(name="opool", bufs=2))
        ps = ctx.enter_context(tc.tile_pool(name="ps", bufs=4, space="PSUM"))
        psc = ctx.enter_context(tc.tile_pool(name="psc", bufs=1, space="PSUM"))

        # constants
        diag_t = consts.tile([128, 128], F32, tag="diag", name="diag_t")
        make_causal_mask(nc, diag_t[:], mask_val=-1e30)   # 0 where q>=k
        diagT_t = consts.tile([128, 128], F32, tag="diagT", name="diagT_t")
        nc.gpsimd.memset(diagT_t[:], 0.0)
        nc.gpsimd.affine_select(   # 0 where k<=q (transposed causal mask)
            out=diagT_t[:], in_=diagT_t[:], compare_op=mybir.AluOpType.is_ge,
            fill=-1e30, base=0, pattern=[[1, 128]], channel_multiplier=-1)
        ones_f = consts.tile([1, 128], F32, tag="ones_f", name="ones_f")
        nc.gpsimd.memset(ones_f[:], 1.0)
        ones_t = consts.tile([1, 128], RDT, tag="ones", name="ones_t")
        nc.vector.tensor_copy(ones_t[:], ones_f[:])
        onescol_f = consts.tile([128, QT * HPC], F32, tag="onescol", name="onescol_f")
        nc.gpsimd.memset(onescol_f[:], 1.0)
        zerof_t = consts.tile([128, 512], F32, tag="zerof", name="zerof_t")
        nc.gpsimd.memset(zerof_t[:], 0.0)
        zeror_t = consts.tile([128, 512], RDT, tag="zeror", name="zeror_t")
        nc.vector.tensor_copy(zeror_t[:], zerof_t[:])

        # weights (DMA'd straight into matmul operand dtype)
        wq_t = wpool.tile([128, CK, HD], RDT, tag="wq", name="wq_t")
        wk_t = wpool.tile([128, CK, HD], RDT, tag="wk", name="wk_t")
        wkv_t = wpool.tile([128, CK, HD], RDT, tag="wkv", name="wkv_t")
        wo_t = wpool.tile([128, 2, C], RDT, tag="wo", name="wo_t")
        nc.sync.dma_start(wq_t[:], wq.rearrange("(ck p) d -> p ck d", p=128).bitcast(RDT))
        nc.sync.dma_start(wk_t[:], wk.rearrange("(ck p) d -> p ck d", p=128).bitcast(RDT))
        nc.sync.dma_start(wkv_t[:], wkv.rearrange("(ck p) d -> p ck d", p=128).bitcast(RDT))
        nc.sync.dma_start(wo_t[:], wo.rearrange("(dc p) c -> p dc c", p=128).bitcast(RDT))
        bq_t = consts.tile([128, 2, 1], F32, tag="bq", name="bq_t")
        bk_t = consts.tile([128, 2, 1], F32, tag="bk", name="bk_t")
        bkv_t = consts.tile([1, HD], RDT, tag="bkv", name="bkv_t")
        nc.sync.dma_start(bq_t[:], bq.rearrange("m p one -> p m one"))
        nc.sync.dma_start(bk_t[:], bk.rearrange("m p one -> p m one"))
        nc.sync.dma_start(bkv_t[:], bkv.bitcast(RDT))

        # persistent activations (matmul operands -> RDT)
        qpT_t = [acts.tile([128, T], RDT, tag=f"qpT{m}", name=f"qpT{m}")
                 for m in range(2)]
        kpT_t = [acts.tile([128, T], RDT, tag=f"kpT{m}", name=f"kpT{m}")
                 for m in range(2)]
        # per-head v columns plus ones column for the denominator row
        vp_t = acts.tile([128, QT, HPC, D + 1], RDT, tag="vp", name="vp_t")
        nc.vector.tensor_copy(
            vp_t[:, :, :, D:D + 1],
            onescol_f[:].rearrange("p (a b) -> p a b", a=QT).unsqueeze(3))
        ctxT_t = [acts.tile([128, T], RDT, tag=f"ctxT{m}", name=f"ctxT{m}")
                  for m in range(2)]

        # ---- Phase 1: projections (stream qT/kT in halves of T) ----
        qT_r = qT.rearrange("(ck p) t -> p ck t", p=128).bitcast(RDT)
        kT_r = kT.rearrange("(ck p) t -> p ck t", p=128).bitcast(RDT)
        TH = T // 2
        for th in range(2):
            xh = big.tile([128, CK, TH], RDT, tag="xT", name="xh")
            nc.sync.dma_start(xh[:], qT_r[:, :, th * TH:(th + 1) * TH])
            for m in range(2):
                for tc512 in range(TH // 512):
                    p = ps.tile([128, 512], F32, tag="pproj", name="pproj")
                    for ck in range(CK):
                        nc.tensor.matmul(
                            p[:], wq_t[:, ck, m * 128:(m + 1) * 128],
                            xh[:, ck, tc512 * 512:(tc512 + 1) * 512],
                            start=(ck == 0), stop=(ck == CK - 1))
                    nc.scalar.activation(
                        qpT_t[m][:, th * TH + tc512 * 512:th * TH + (tc512 + 1) * 512],
                        p[:], ACT.Identity, bias=bq_t[:, m, :])
        for th in range(2):
            xh = big.tile([128, CK, TH], RDT, tag="xT", name="xh")
            nc.sync.dma_start(xh[:], kT_r[:, :, th * TH:(th + 1) * TH])
            for m in range(2):
                for tc512 in range(TH // 512):
                    p = ps.tile([128, 512], F32, tag="pproj", name="pproj")
                    for ck in range(CK):
                        nc.tensor.matmul(
                            p[:], wk_t[:, ck, m * 128:(m + 1) * 128],
                            xh[:, ck, tc512 * 512:(tc512 + 1) * 512],
                            start=(ck == 0), stop=(ck == CK - 1))
                    nc.scalar.activation(
                        kpT_t[m][:, th * TH + tc512 * 512:th * TH + (tc512 + 1) * 512],
                        p[:], ACT.Identity, bias=bk_t[:, m, :])
            # vp for this half: vp[t,:] = k @ Wkv + bkv (K=1 ones trick for bias)
            for ti in range(th * (QT // 2), (th + 1) * (QT // 2)):
                tl = ti * 128 - th * TH
                p = ps.tile([128, 512], F32, tag="pproj", name="pproj")
                for ck in range(CK):
                    nc.tensor.matmul(
                        p[:, :HD], xh[:, ck, tl:tl + 128], wkv_t[:, ck, :],
                        start=(ck == 0), stop=False)
                nc.tensor.matmul(p[:, :HD], ones_t[:], bkv_t[:],
                                 start=False, stop=True)
                nc.vector.tensor_copy(
                    vp_t[:, ti, :, 0:D],
                    p[:, :HD].rearrange("p (h d) -> p h d", h=HPC))

        # ---- Phase 2: attention per head pair, q-windows of 512 ----
        for hp in range(2):
            for j in range(4):
                # A-pass: per q-tile, S rows -> exp -> normalize -> DMA
                for i in range(4 * j, 4 * j + 4):
                    win = (i + 1) * 128
                    nch = (win + 511) // 512
                    dk = (i * 128) // 512
                    strips = {}
                    pps = {}
                    rs = {}
                    for g in range(2):
                        strips[g] = apool.tile([128, T], F32, tag="astrip",
                                               name="astrip")
                        rs[g] = small.tile([128, 4], F32, tag=f"rs{g}",
                                           name=f"rs{g}")
                    for kc in range(nch):
                        n = min(512, win - kc * 512)
                        for g in range(2):
                            off = 64 * g
                            p = ps.tile([128, 512], F32, tag="pproj",
                                        name="pscore")
                            pps[g] = p
                            nc.tensor.matmul(
                                p[:, :n],
                                qpT_t[hp][off:off + 64, i * 128:(i + 1) * 128],
                                kpT_t[hp][off:off + 64, kc * 512:kc * 512 + n],
                                start=True, stop=True)
                        for g in range(2):
                            p = pps[g]
                            if kc == dk:
                                pos = i * 128 - dk * 512
                                nc.vector.tensor_tensor(
                                    out=p[:, pos:pos + 128],
                                    in0=p[:, pos:pos + 128],
                                    in1=diag_t[:], op=mybir.AluOpType.add)
                            nc.scalar.activation(
                                strips[g][:, kc * 512:kc * 512 + n],
                                p[:, :n], ACT.Exp,
                                accum_out=rs[g][:, kc:kc + 1])
                    # row sums via reduce over the causal window
                    for g in range(2):
                        h = 2 * hp + g
                        rtot = small.tile([128, 1], F32, tag="rtot", name="rtot")
                        nc.vector.tensor_reduce(rtot[:], rs[g][:, :nch],
                                                axis=mybir.AxisListType.X,
                                                op=mybir.AluOpType.add)
                        rcp = small.tile([128, 1], F32, tag="rcp", name="rcp")
                        nc.vector.reciprocal(rcp[:], rtot[:])
                        nc.vector.tensor_scalar_mul(strips[g][:, :win],
                                                    strips[g][:, :win], rcp[:])
                        nc.sync.dma_start(
                            attnw[h, i * 128:(i + 1) * 128, 0:win],
                            strips[g][:, :win])
                # ST-pass + PV: S^T tiles -> exp -> E^T, PV accumulates ctxT
                pcv = {}
                for g in range(2):
                    pcv[g] = psc.tile([D + 1, 512], F32, tag=f"pcv{g}",
                                      name=f"pcv{g}")
                nkc = 4 * j + 4
                for kc in range(nkc):
                    ets = {}
                    pts = {}
                    for g in range(2):
                        off = 64 * g
                        p = ps.tile([128, 512], F32, tag="pproj", name="pst")
                        pts[g] = p
                        nc.tensor.matmul(
                            p[:],
                            kpT_t[hp][off:off + 64, kc * 128:(kc + 1) * 128],
                            qpT_t[hp][off:off + 64, j * 512:(j + 1) * 512],
                            start=True, stop=True)
                    ld = kc - 4 * j   # local diagonal block index
                    for g in range(2):
                        p = pts[g]
                        et = etp.tile([128, 512], RDT, tag="et", name="et")
                        ets[g] = et
                        if ld >= 0:
                            nc.vector.tensor_tensor(
                                out=p[:, ld * 128:(ld + 1) * 128],
                                in0=p[:, ld * 128:(ld + 1) * 128],
                                in1=diagT_t[:], op=mybir.AluOpType.add)
                            if ld > 0:
                                nc.vector.tensor_copy(et[:, :ld * 128],
                                                      zeror_t[:, :ld * 128])
                            nc.scalar.activation(et[:, ld * 128:],
                                                 p[:, ld * 128:], ACT.Exp)
                        else:
                            nc.scalar.activation(et[:], p[:], ACT.Exp)
                    for g in range(2):
                        h = 2 * hp + g
                        nc.tensor.matmul(pcv[g][:], vp_t[:, kc, h, :], ets[g][:],
                                         start=(kc == 0), stop=(kc == nkc - 1))
                for g in range(2):
                    h = 2 * hp + g
                    off = 64 * g
                    # reciprocal of the denominator row, broadcast via K=1 matmul
                    rT = small.tile([1, 512], RDT, tag="rT", name="rT")
                    with nc.allow_low_precision(reason="fp32r denom reciprocal"):
                        nc.vector.reciprocal(rT[:], pcv[g][D:D + 1, :])
                    pb = psc.tile([D, 512], F32, tag="pb", name="pb")
                    nc.tensor.matmul(pb[:], ones_t[0:1, 0:D], rT[:],
                                     start=True, stop=True)
                    pb_sb = small.tile([D, 512], F32, tag="pbsb", name="pb_sb")
                    nc.any.tensor_copy(pb_sb[:], pb[:])
                    nc.vector.tensor_tensor(
                        out=ctxT_t[hp][off:off + 64, j * 512:(j + 1) * 512],
                        in0=pcv[g][0:D, :], in1=pb_sb[:],
                        op=mybir.AluOpType.mult)

        # ---- Phase 3: output projection (partial; host sums across cores) ----
        for i in range(QT):
            o_sb = opool.tile([128, C], F32, tag="osb", name="o_sb")
            for n2 in range(2):
                p = ps.tile([128, 512], F32, tag="pproj", name="pout")
                for dc in range(2):
                    nc.tensor.matmul(p[:],
                                     ctxT_t[dc][:, i * 128:(i + 1) * 128],
                                     wo_t[:, dc, n2 * 512:(n2 + 1) * 512],
                                     start=(dc == 0), stop=(dc == 1))
                nc.any.tensor_copy(o_sb[:, n2 * 512:(n2 + 1) * 512], p[:])
            nc.sync.dma_start(outp[i * 128:(i + 1) * 128, :], o_sb[:])

    nc.compile()
    return nc


_cached = {}


def _get_prog():
    if "nc" not in _cached:
        _cached["nc"] = _build()
    return _cached["nc"]


def _prep_inputs(q, k, Wq, bq, Wk, bk, Wv, bv, Wo, bo, mask):
    """Build the 8 per-core input maps (host-side sharding)."""
    q = np.asarray(q, np.float32)
    k = np.asarray(k, np.float32)
    Wq = np.asarray(Wq, np.float64)
    Wk = np.asarray(Wk, np.float64)
    Wv = np.asarray(Wv, np.float64)
    Wo = np.asarray(Wo, np.float64)
    bq = np.asarray(bq, np.float64)
    bk = np.asarray(bk, np.float64)
    bv = np.asarray(bv, np.float64)
    scale = 1.0 / math.sqrt(D)
    in_maps = []
    for core in range(NCORES):
        b = core // 4
        h0 = HPC * (core % 4)
        sl = slice(h0 * D, (h0 + HPC) * D)
        in_maps.append({
            "qT": np.ascontiguousarray(q[b].T),
            "kT": np.ascontiguousarray(k[b].T),
            "wq": np.ascontiguousarray(Wq[:, sl] * scale).astype(np.float32),
            "wk": np.ascontiguousarray(Wk[:, sl]).astype(np.float32),
            "wkv": np.ascontiguousarray(Wk @ Wv[:, sl]).astype(np.float32),
            "wo": np.ascontiguousarray(Wo[sl, :]).astype(np.float32),
            "bq": (bq[sl] * scale).astype(np.float32).reshape(2, 128, 1),
            "bk": bk[sl].astype(np.float32).reshape(2, 128, 1),
            "bkv": (bk @ Wv[:, sl] + bv[sl]).astype(np.float32).reshape(1, HD),
        })
    return in_maps


def _run(inputs, trace=False, trace_kwargs=None):
    nc = _get_prog()
    in_maps = _prep_inputs(**inputs)
    res = run_bass_kernel_spmd(nc, in_maps, list(range(NCORES)), trace=trace,
                               **(trace_kwargs or {}))
    bo = np.asarray(inputs["bo"], np.float64)
    out = np.empty((B, T, C), np.float32)
    attnw = np.zeros((B, H, T, T), np.float32)
    for b in range(B):
        acc = np.zeros((T, C), np.float64)
        for cc in range(4):
            core = b * 4 + cc
            acc += res.results[core]["outp"]
            dev = res.results[core]["attnw"]
            dst = attnw[b, cc * HPC:(cc + 1) * HPC]
            # device wrote only the causal part; upper triangle stays zero
            for i in range(QT):
                win = (i + 1) * 128
                dst[:, i * 128:(i + 1) * 128, :win] = \
                    dev[:, i * 128:(i + 1) * 128, :win]
        out[b] = (acc + bo).astype(np.float32)
    return (out, attnw), res


def kernel(**inputs):
    (out, attnw), _ = _run(inputs, trace=False)
    return out, attnw


# revision 12
# speedup vs baseline: 1.1606x; 1.1606x over previous
"""Trainium2 Bass kernel for causal multi-head attention (B=2, T=2048, C=1024, H=16).

Reference semantics:
    qp = q @ Wq + bq ; kp = k @ Wk + bk ; vp = kp @ Wv + bv   (V from projected K)
    S  = (qh @ khT) / sqrt(D), causal mask, A = softmax(S)
    ctx = A @ vh ; out = ctx @ Wo + bo
Returns (out, attention_weights).

Sharding: 8 cores; core c handles batch b = c//4 and 4 heads h0 = 4*(c%4).
Each core gets transposed activations qT/kT [C, T], head-sliced weights, and
host-folded Wkv = Wk @ Wv[:, slice] so vp comes straight from k. The scores
scale 1/sqrt(D) is folded into Wq/bq. Partial output projections are summed on
the host (the all-reduce of the sharding strategy); bo and the upper-triangle
zeros of attention_weights are applied on the host as well.

Device data flow per core (all matmul operands float32r ~ 12-bit mantissa):
  qpT/kpT [256, T] (features on partitions), vp65 [T, 4, 65] (per-head v
  columns + a ones column that makes the PV matmul also produce the softmax
  denominator row).
  Per head pair (partition offsets 0/64 -> concurrent PE row groups) and
  q-window of 512:
    A-pass: S = qh^T-tile @ kh window (K=64 matmuls, paired), additive causal
    mask on the diagonal block, exp on ScalarE with fused row-sum accum,
    normalize by reciprocal row sum, DMA the causal part of A out.
    ST-pass: S^T tiles [k-tile, q-window] straight from kpT/qpT (paired),
    exp -> E^T (unnormalized), PV matmul ctxT[65, 512] += vp65^T @ E^T whose
    row 64 is the denominator; ctxT rows 0..63 are multiplied by the
    broadcast reciprocal denominator (K=1 ones outer product on the PE).
  Output projection from ctxT with the Wo slice.
"""
import math
import numpy as np
from contextlib import ExitStack

import concourse.mybir as mybir
import concourse.tile as tile
from concourse import bacc
from concourse.bass_utils import run_bass_kernel_spmd
from concourse.masks import make_causal_mask

B, T, C, H = 2, 2048, 1024, 16
D = C // H            # 64
NCORES = 8
HPC = 4               # heads per core
HD = HPC * D          # 256
CK = C // 128         # 8 contraction chunks
QT = T // 128         # 16 q tiles
F32 = mybir.dt.float32
RDT = mybir.dt.float32r   # matmul operand dtype

ACT = mybir.ActivationFunctionType


def _build():
    nc = bacc.Bacc("TRN2", target_bir_lowering=False, debug=False,
                   num_devices=NCORES)
    qT = nc.dram_tensor("qT", [C, T], F32, kind="ExternalInput").ap()
    kT = nc.dram_tensor("kT", [C, T], F32, kind="ExternalInput").ap()
    wq = nc.dram_tensor("wq", [C, HD], F32, kind="ExternalInput").ap()
    wk = nc.dram_tensor("wk", [C, HD], F32, kind="ExternalInput").ap()
    wkv = nc.dram_tensor("wkv", [C, HD], F32, kind="ExternalInput").ap()
    wo = nc.dram_tensor("wo", [HD, C], F32, kind="ExternalInput").ap()
    bq = nc.dram_tensor("bq", [2, 128, 1], F32, kind="ExternalInput").ap()
    bk = nc.dram_tensor("bk", [2, 128, 1], F32, kind="ExternalInput").ap()
    bkv = nc.dram_tensor("bkv", [1, HD], F32, kind="ExternalInput").ap()
    attnw = nc.dram_tensor("attnw", [HPC, T, T], F32, kind="ExternalOutput").ap()
    outp = nc.dram_tensor("outp", [T, C], F32, kind="ExternalOutput").ap()

    with tile.TileContext(nc) as tc, ExitStack() as ctx:
        consts = ctx.enter_context(tc.tile_pool(name="consts", bufs=1))
        wpool = ctx.enter_context(tc.tile_pool(name="wpool", bufs=1))
        big = ctx.enter_context(tc.tile_pool(name="big", bufs=1))
        acts = ctx.enter_context(tc.tile_pool(name="acts", bufs=1))
        apool = ctx.enter_context(tc.tile_pool(name="apool", bufs=4))
        etp = ctx.enter_context(tc.tile_pool(name="etp", bufs=6))
        small = ctx.enter_context(tc.tile_pool(name="small", bufs=4))
        opool = ctx.enter_context(tc.tile_pool(name="opool", bufs=2))
        ps = ctx.enter_context(tc.tile_pool(name="ps", bufs=3, space="PSUM"))
        psc = ctx.enter_context(tc.tile_pool(name="psc", bufs=1, space="PSUM"))

        # constants
        diag_t = consts.tile([128, 128], F32, tag="diag", name="diag_t")
        make_causal_mask(nc, diag_t[:], mask_val=-1e30)   # 0 where q>=k
        diagT_t = consts.tile([128, 128], F32, tag="diagT", name="diagT_t")
        nc.gpsimd.memset(diagT_t[:], 0.0)
        nc.gpsimd.affine_select(   # 0 where k<=q (transposed causal mask)
            out=diagT_t[:], in_=diagT_t[:], compare_op=mybir.AluOpType.is_ge,
            fill=-1e30, base=0, pattern=[[1, 128]], channel_multiplier=-1)
        ones_f = consts.tile([1, 128], F32, tag="ones_f", name="ones_f")
        nc.gpsimd.memset(ones_f[:], 1.0)
        ones_t = consts.tile([1, 128], RDT, tag="ones", name="ones_t")
        nc.vector.tensor_copy(ones_t[:], ones_f[:])
        onescol_f = consts.tile([128, QT * HPC], F32, tag="onescol", name="onescol_f")
        nc.gpsimd.memset(onescol_f[:], 1.0)
        zerof_t = consts.tile([128, 512], F32, tag="zerof", name="zerof_t")
        nc.gpsimd.memset(zerof_t[:], 0.0)
        zeror_t = consts.tile([128, 512], RDT, tag="zeror", name="zeror_t")
        nc.vector.tensor_copy(zeror_t[:], zerof_t[:])

        # weights (DMA'd straight into matmul operand dtype)
        wq_t = wpool.tile([128, CK, HD], RDT, tag="wq", name="wq_t")
        wk_t = wpool.tile([128, CK, HD], RDT, tag="wk", name="wk_t")
        wkv_t = wpool.tile([128, CK, HD], RDT, tag="wkv", name="wkv_t")
        wo_t = wpool.tile([128, 2, C], RDT, tag="wo", name="wo_t")
        nc.sync.dma_start(wq_t[:], wq.rearrange("(ck p) d -> p ck d", p=128).bitcast(RDT))
        nc.sync.dma_start(wk_t[:], wk.rearrange("(ck p) d -> p ck d", p=128).bitcast(RDT))
        nc.sync.dma_start(wkv_t[:], wkv.rearrange("(ck p) d -> p ck d", p=128).bitcast(RDT))
        nc.sync.dma_start(wo_t[:], wo.rearrange("(dc p) c -> p dc c", p=128).bitcast(RDT))
        bq_t = consts.tile([128, 2, 1], F32, tag="bq", name="bq_t")
        bk_t = consts.tile([128, 2, 1], F32, tag="bk", name="bk_t")
        bkv_t = consts.tile([1, HD], RDT, tag="bkv", name="bkv_t")
        nc.sync.dma_start(bq_t[:], bq.rearrange("m p one -> p m one"))
        nc.sync.dma_start(bk_t[:], bk.rearrange("m p one -> p m one"))
        nc.sync.dma_start(bkv_t[:], bkv.bitcast(RDT))

        # persistent activations (matmul operands -> RDT)
        qpT_t = [acts.tile([128, T], RDT, tag=f"qpT{m}", name=f"qpT{m}")
                 for m in range(2)]
        kpT_t = [acts.tile([128, T], RDT, tag=f"kpT{m}", name=f"kpT{m}")
                 for m in range(2)]
        # per-head v columns plus ones column for the denominator row
        vp_t = acts.tile([128, QT, HPC, D + 1], RDT, tag="vp", name="vp_t")
        nc.vector.tensor_copy(
            vp_t[:, :, :, D:D + 1],
            onescol_f[:].rearrange("p (a b) -> p a b", a=QT).unsqueeze(3))
        ctxT_t = [acts.tile([128, T], RDT, tag=f"ctxT{m}", name=f"ctxT{m}")
                  for m in range(2)]

        # ---- Phase 1: projections (stream qT/kT in halves of T) ----
        qT_r = qT.rearrange("(ck p) t -> p ck t", p=128).bitcast(RDT)
        kT_r = kT.rearrange("(ck p) t -> p ck t", p=128).bitcast(RDT)
        TH = T // 2
        for th in range(2):
            xh = big.tile([128, CK, TH], RDT, tag="xT", name="xh")
            nc.sync.dma_start(xh[:], qT_r[:, :, th * TH:(th + 1) * TH])
            for m in range(2):
                for tc512 in range(TH // 512):
                    p = ps.tile([128, 512], F32, tag="pscore", name="pproj")
                    for ck in range(CK):
                        nc.tensor.matmul(
                            p[:], wq_t[:, ck, m * 128:(m + 1) * 128],
                            xh[:, ck, tc512 * 512:(tc512 + 1) * 512],
                            start=(ck == 0), stop=(ck == CK - 1))
                    nc.vector.tensor_scalar_add(
                        qpT_t[m][:, th * TH + tc512 * 512:th * TH + (tc512 + 1) * 512],
                        p[:], bq_t[:, m, :])
        for th in range(2):
            xh = big.tile([128, CK, TH], RDT, tag="xT", name="xh")
            nc.sync.dma_start(xh[:], kT_r[:, :, th * TH:(th + 1) * TH])
            for m in range(2):
                for tc512 in range(TH // 512):
                    p = ps.tile([128, 512], F32, tag="pscore", name="pproj")
                    for ck in range(CK):
                        nc.tensor.matmul(
                            p[:], wk_t[:, ck, m * 128:(m + 1) * 128],
                            xh[:, ck, tc512 * 512:(tc512 + 1) * 512],
                            start=(ck == 0), stop=(ck == CK - 1))
                    nc.vector.tensor_scalar_add(
                        kpT_t[m][:, th * TH + tc512 * 512:th * TH + (tc512 + 1) * 512],
                        p[:], bk_t[:, m, :])
            # vp for this half: vp[t,:] = k @ Wkv + bkv (K=1 ones trick for bias)
            for ti in range(th * (QT // 2), (th + 1) * (QT // 2)):
                tl = ti * 128 - th * TH
                p = ps.tile([128, 512], F32, tag="pscore", name="pproj")
                for ck in range(CK):
                    nc.tensor.matmul(
                        p[:, :HD], xh[:, ck, tl:tl + 128], wkv_t[:, ck, :],
                        start=(ck == 0), stop=False)
                nc.tensor.matmul(p[:, :HD], ones_t[:], bkv_t[:],
                                 start=False, stop=True)
                nc.vector.tensor_copy(
                    vp_t[:, ti, :, 0:D],
                    p[:, :HD].rearrange("p (h d) -> p h d", h=HPC))

        # ---- Phase 2: attention per head pair, q-windows of 512 ----
        for hp in range(2):
            for j in range(4):
                # A-pass: per q-tile, S rows -> exp -> normalize -> DMA
                for i in range(4 * j, 4 * j + 4):
                    win = (i + 1) * 128
                    nch = (win + 511) // 512
                    dk = (i * 128) // 512
                    strips = {}
                    rs = {}
                    for g in range(2):
                        strips[g] = apool.tile([128, T], F32, tag="astrip",
                                               name="astrip")
                        rs[g] = small.tile([128, 4], F32, tag=f"rs{g}",
                                           name=f"rs{g}")
                    for g in range(2):
                        off = 64 * g
                        for kc in range(nch):
                            n = min(512, win - kc * 512)
                            p = ps.tile([128, 512], F32, tag="pscore",
                                        name="pscore")
                            nc.tensor.matmul(
                                p[:, :n],
                                qpT_t[hp][off:off + 64, i * 128:(i + 1) * 128],
                                kpT_t[hp][off:off + 64, kc * 512:kc * 512 + n],
                                start=True, stop=True)
                            if kc == dk:
                                pos = i * 128 - dk * 512
                                nc.vector.tensor_tensor(
                                    out=p[:, pos:pos + 128],
                                    in0=p[:, pos:pos + 128],
                                    in1=diag_t[:], op=mybir.AluOpType.add)
                            nc.scalar.activation(
                                strips[g][:, kc * 512:kc * 512 + n],
                                p[:, :n], ACT.Exp,
                                accum_out=rs[g][:, kc:kc + 1])
                    # row sums via reduce over the causal window
                    for g in range(2):
                        h = 2 * hp + g
                        rtot = small.tile([128, 1], F32, tag="rtot", name="rtot")
                        nc.vector.tensor_reduce(rtot[:], rs[g][:, :nch],
                                                axis=mybir.AxisListType.X,
                                                op=mybir.AluOpType.add)
                        rcp = small.tile([128, 1], F32, tag="rcp", name="rcp")
                        nc.vector.reciprocal(rcp[:], rtot[:])
                        nc.vector.tensor_scalar_mul(strips[g][:, :win],
                                                    strips[g][:, :win], rcp[:])
                        nc.sync.dma_start(
                            attnw[h, i * 128:(i + 1) * 128, 0:win],
                            strips[g][:, :win])
                # ST-pass + PV: S^T tiles -> exp -> E^T, PV accumulates ctxT
                pcv = {}
                for g in range(2):
                    pcv[g] = psc.tile([D + 1, 512], F32, tag=f"pcv{g}",
                                      name=f"pcv{g}")
                nkc = 4 * j + 4
                for kc in range(nkc):
                    ets = {}
                    pts = {}
                    for g in range(2):
                        off = 64 * g
                        p = ps.tile([128, 512], F32, tag="pst", name="pst", bufs=2)
                        pts[g] = p
                        nc.tensor.matmul(
                            p[:],
                            kpT_t[hp][off:off + 64, kc * 128:(kc + 1) * 128],
                            qpT_t[hp][off:off + 64, j * 512:(j + 1) * 512],
                            start=True, stop=True)
                    ld = kc - 4 * j   # local diagonal block index
                    for g in range(2):
                        p = pts[g]
                        et = etp.tile([128, 512], RDT, tag="et", name="et")
                        ets[g] = et
                        if ld >= 0:
                            nc.vector.tensor_tensor(
                                out=p[:, ld * 128:(ld + 1) * 128],
                                in0=p[:, ld * 128:(ld + 1) * 128],
                                in1=diagT_t[:], op=mybir.AluOpType.add)
                            if ld > 0:
                                nc.vector.tensor_copy(et[:, :ld * 128],
                                                      zeror_t[:, :ld * 128])
                            nc.scalar.activation(et[:, ld * 128:],
                                                 p[:, ld * 128:], ACT.Exp)
                        else:
                            nc.scalar.activation(et[:], p[:], ACT.Exp)
                    for g in range(2):
                        h = 2 * hp + g
                        nc.tensor.matmul(pcv[g][:], vp_t[:, kc, h, :], ets[g][:],
                                         start=(kc == 0), stop=(kc == nkc - 1))
                for g in range(2):
                    h = 2 * hp + g
                    off = 64 * g
                    # reciprocal of the denominator row, broadcast via K=1 matmul
                    rT = small.tile([1, 512], RDT, tag="rT", name="rT")
                    with nc.allow_low_precision(reason="fp32r denom reciprocal"):
                        nc.vector.reciprocal(rT[:], pcv[g][D:D + 1, :])
                    pb = psc.tile([D, 512], F32, tag="pb", name="pb")
                    nc.tensor.matmul(pb[:], ones_t[0:1, 0:D], rT[:],
                                     start=True, stop=True)
                    pb_sb = small.tile([D, 512], F32, tag="pbsb", name="pb_sb")
                    nc.any.tensor_copy(pb_sb[:], pb[:])
                    nc.vector.tensor_tensor(
                        out=ctxT_t[hp][off:off + 64, j * 512:(j + 1) * 512],
                        in0=pcv[g][0:D, :], in1=pb_sb[:],
                        op=mybir.AluOpType.mult)

        # ---- Phase 3: output projection (partial; host sums across cores) ----
        for i in range(QT):
            o_sb = opool.tile([128, C], F32, tag="osb", name="o_sb")
            for n2 in range(2):
                p = ps.tile([128, 512], F32, tag="pscore", name="pout")
                for dc in range(2):
                    nc.tensor.matmul(p[:],
                                     ctxT_t[dc][:, i * 128:(i + 1) * 128],
                                     wo_t[:, dc, n2 * 512:(n2 + 1) * 512],
                                     start=(dc == 0), stop=(dc == 1))
                nc.vector.tensor_copy(o_sb[:, n2 * 512:(n2 + 1) * 512], p[:])
            nc.sync.dma_start(outp[i * 128:(i + 1) * 128, :], o_sb[:])

    nc.compile()
    return nc


_cached = {}


def _get_prog():
    if "nc" not in _cached:
        _cached["nc"] = _build()
    return _cached["nc"]


def _prep_inputs(q, k, Wq, bq, Wk, bk, Wv, bv, Wo, bo, mask):
    """Build the 8 per-core input maps (host-side sharding)."""
    q = np.asarray(q, np.float32)
    k = np.asarray(k, np.float32)
    Wq = np.asarray(Wq, np.float64)
    Wk = np.asarray(Wk, np.float64)
    Wv = np.asarray(Wv, np.float64)
    Wo = np.asarray(Wo, np.float64)
    bq = np.asarray(bq, np.float64)
    bk = np.asarray(bk, np.float64)
    bv = np.asarray(bv, np.float64)
    scale = 1.0 / math.sqrt(D)
    in_maps = []
    for core in range(NCORES):
        b = core // 4
        h0 = HPC * (core % 4)
        sl = slice(h0 * D, (h0 + HPC) * D)
        in_maps.append({
            "qT": np.ascontiguousarray(q[b].T),
            "kT": np.ascontiguousarray(k[b].T),
            "wq": np.ascontiguousarray(Wq[:, sl] * scale).astype(np.float32),
            "wk": np.ascontiguousarray(Wk[:, sl]).astype(np.float32),
            "wkv": np.ascontiguousarray(Wk @ Wv[:, sl]).astype(np.float32),
            "wo": np.ascontiguousarray(Wo[sl, :]).astype(np.float32),
            "bq": (bq[sl] * scale).astype(np.float32).reshape(2, 128, 1),
            "bk": bk[sl].astype(np.float32).reshape(2, 128, 1),
            "bkv": (bk @ Wv[:, sl] + bv[sl]).astype(np.float32).reshape(1, HD),
        })
    return in_maps


def _run(inputs, trace=False, trace_kwargs=None):
    nc = _get_prog()
    in_maps = _prep_inputs(**inputs)
    res = run_bass_kernel_spmd(nc, in_maps, list(range(NCORES)), trace=trace,
                               **(trace_kwargs or {}))
    bo = np.asarray(inputs["bo"], np.float64)
    out = np.empty((B, T, C), np.float32)
    attnw = np.zeros((B, H, T, T), np.float32)
    for b in range(B):
        acc = np.zeros((T, C), np.float64)
        for cc in range(4):
            core = b * 4 + cc
            acc += res.results[core]["outp"]
            dev = res.results[core]["attnw"]
            dst = attnw[b, cc * HPC:(cc + 1) * HPC]
            # device wrote only the causal part; upper triangle stays zero
            for i in range(QT):
                win = (i + 1) * 128
                dst[:, i * 128:(i + 1) * 128, :win] = \
                    dev[:, i * 128:(i + 1) * 128, :win]
        out[b] = (acc + bo).astype(np.float32)
    return (out, attnw), res


def kernel(**inputs):
    (out, attnw), _ = _run(inputs, trace=False)
    return out, attnw


# revision 16
# speedup vs baseline: 1.1924x; 1.0274x over previous
"""Trainium2 Bass kernel for causal multi-head attention (B=2, T=2048, C=1024, H=16).

Reference semantics:
    qp = q @ Wq + bq ; kp = k @ Wk + bk ; vp = kp @ Wv + bv   (V from projected K)
    S  = (qh @ khT) / sqrt(D), causal mask, A = softmax(S)
    ctx = A @ vh ; out = ctx @ Wo + bo
Returns (out, attention_weights).

Sharding: 8 cores; core c handles batch b = c//4 and 4 heads h0 = 4*(c%4).
Each core gets transposed activations qT/kT [C, T], head-sliced weights, and
host-folded Wkv = Wk @ Wv[:, slice] so vp comes straight from k. The scores
scale 1/sqrt(D) is folded into Wq/bq. Partial output projections are summed on
the host (the all-reduce of the sharding strategy); bo and the upper-triangle
zeros of attention_weights are applied on the host as well.

Device data flow per core (all matmul operands float32r ~ 12-bit mantissa):
  qpT/kpT [256, T] (features on partitions), vp65 [T, 4, 65] (per-head v
  columns + a ones column that makes the PV matmul also produce the softmax
  denominator row).
  Per head pair (partition offsets 0/64 -> concurrent PE row groups) and
  q-window of 512:
    A-pass: S = qh^T-tile @ kh window (K=64 matmuls, paired), additive causal
    mask on the diagonal block, exp on ScalarE with fused row-sum accum,
    normalize by reciprocal row sum, DMA the causal part of A out.
    ST-pass: S^T tiles [k-tile, q-window] straight from kpT/qpT (paired),
    exp -> E^T (unnormalized), PV matmul ctxT[65, 512] += vp65^T @ E^T whose
    row 64 is the denominator; ctxT rows 0..63 are multiplied by the
    broadcast reciprocal denominator (K=1 ones outer product on the PE).
  Output projection from ctxT with the Wo slice.
"""
import math
import numpy as np
from contextlib import ExitStack

import concourse.mybir as mybir
import concourse.tile as tile
from concourse import bacc
from concourse.bass_utils import run_bass_kernel_spmd
from concourse.masks import make_causal_mask

B, T, C, H = 2, 2048, 1024, 16
D = C // H            # 64
NCORES = 8
HPC = 4               # heads per core
HD = HPC * D          # 256
CK = C // 128         # 8 contraction chunks
QT = T // 128         # 16 q tiles
F32 = mybir.dt.float32
RDT = mybir.dt.float32r   # matmul operand dtype

ACT = mybir.ActivationFunctionType


def _build():
    nc = bacc.Bacc("TRN2", target_bir_lowering=False, debug=False,
                   num_devices=NCORES)
    qT = nc.dram_tensor("qT", [C, T], F32, kind="ExternalInput").ap()
    kT = nc.dram_tensor("kT", [C, T], F32, kind="ExternalInput").ap()
    wq = nc.dram_tensor("wq", [C, HD], F32, kind="ExternalInput").ap()
    wk = nc.dram_tensor("wk", [C, HD], F32, kind="ExternalInput").ap()
    wkv = nc.dram_tensor("wkv", [C, HD], F32, kind="ExternalInput").ap()
    wo = nc.dram_tensor("wo", [HD, C], F32, kind="ExternalInput").ap()
    bq = nc.dram_tensor("bq", [2, 128, 1], F32, kind="ExternalInput").ap()
    bk = nc.dram_tensor("bk", [2, 128, 1], F32, kind="ExternalInput").ap()
    bkv = nc.dram_tensor("bkv", [1, HD], F32, kind="ExternalInput").ap()
    attnw = nc.dram_tensor("attnw", [HPC, T, T], F32, kind="ExternalOutput").ap()
    outp = nc.dram_tensor("outp", [T, C], F32, kind="ExternalOutput").ap()

    with tile.TileContext(nc) as tc, ExitStack() as ctx:
        consts = ctx.enter_context(tc.tile_pool(name="consts", bufs=1))
        wpool = ctx.enter_context(tc.tile_pool(name="wpool", bufs=1))
        big = ctx.enter_context(tc.tile_pool(name="big", bufs=2))
        acts = ctx.enter_context(tc.tile_pool(name="acts", bufs=1))
        apool = ctx.enter_context(tc.tile_pool(name="apool", bufs=4))
        etp = ctx.enter_context(tc.tile_pool(name="etp", bufs=5))
        small = ctx.enter_context(tc.tile_pool(name="small", bufs=4))
        opool = ctx.enter_context(tc.tile_pool(name="opool", bufs=2))
        ps = ctx.enter_context(tc.tile_pool(name="ps", bufs=3, space="PSUM"))
        psc = ctx.enter_context(tc.tile_pool(name="psc", bufs=1, space="PSUM"))

        # constants
        diag_t = consts.tile([128, 128], F32, tag="diag", name="diag_t")
        make_causal_mask(nc, diag_t[:], mask_val=-1e30)   # 0 where q>=k
        diagT_t = consts.tile([128, 128], F32, tag="diagT", name="diagT_t")
        nc.gpsimd.memset(diagT_t[:], 0.0)
        nc.gpsimd.affine_select(   # 0 where k<=q (transposed causal mask)
            out=diagT_t[:], in_=diagT_t[:], compare_op=mybir.AluOpType.is_ge,
            fill=-1e30, base=0, pattern=[[1, 128]], channel_multiplier=-1)
        ones_f = consts.tile([1, 128], F32, tag="ones_f", name="ones_f")
        nc.gpsimd.memset(ones_f[:], 1.0)
        ones_t = consts.tile([1, 128], RDT, tag="ones", name="ones_t")
        nc.vector.tensor_copy(ones_t[:], ones_f[:])
        onescol_f = consts.tile([128, QT * HPC], F32, tag="onescol", name="onescol_f")
        nc.gpsimd.memset(onescol_f[:], 1.0)
        zerof_t = consts.tile([128, 384], F32, tag="zerof", name="zerof_t")
        nc.gpsimd.memset(zerof_t[:], 0.0)
        zeror_t = consts.tile([128, 384], RDT, tag="zeror", name="zeror_t")
        nc.vector.tensor_copy(zeror_t[:], zerof_t[:])

        # weights (DMA'd straight into matmul operand dtype)
        wq_t = wpool.tile([128, CK, HD], RDT, tag="wq", name="wq_t")
        wk_t = wpool.tile([128, CK, HD], RDT, tag="wk", name="wk_t")
        wkv_t = wpool.tile([128, CK, HD], RDT, tag="wkv", name="wkv_t")
        wo_t = wpool.tile([128, 2, C], RDT, tag="wo", name="wo_t")
        nc.sync.dma_start(wq_t[:], wq.rearrange("(ck p) d -> p ck d", p=128).bitcast(RDT))
        nc.sync.dma_start(wk_t[:], wk.rearrange("(ck p) d -> p ck d", p=128).bitcast(RDT))
        nc.sync.dma_start(wkv_t[:], wkv.rearrange("(ck p) d -> p ck d", p=128).bitcast(RDT))
        nc.sync.dma_start(wo_t[:], wo.rearrange("(dc p) c -> p dc c", p=128).bitcast(RDT))
        bq_t = consts.tile([128, 2, 1], F32, tag="bq", name="bq_t")
        bk_t = consts.tile([128, 2, 1], F32, tag="bk", name="bk_t")
        bkv_t = consts.tile([1, HD], RDT, tag="bkv", name="bkv_t")
        nc.sync.dma_start(bq_t[:], bq.rearrange("m p one -> p m one"))
        nc.sync.dma_start(bk_t[:], bk.rearrange("m p one -> p m one"))
        nc.sync.dma_start(bkv_t[:], bkv.bitcast(RDT))

        # persistent activations (matmul operands -> RDT)
        qpT_t = [acts.tile([128, T], RDT, tag=f"qpT{m}", name=f"qpT{m}")
                 for m in range(2)]
        kpT_t = [acts.tile([128, T], RDT, tag=f"kpT{m}", name=f"kpT{m}")
                 for m in range(2)]
        # per-head v columns plus ones column for the denominator row
        vp_t = acts.tile([128, QT, HPC, D + 1], RDT, tag="vp", name="vp_t")
        nc.vector.tensor_copy(
            vp_t[:, :, :, D:D + 1],
            onescol_f[:].rearrange("p (a b) -> p a b", a=QT).unsqueeze(3))
        ctxT_t = [acts.tile([128, T], RDT, tag=f"ctxT{m}", name=f"ctxT{m}")
                  for m in range(2)]

        # ---- Phase 1: projections (stream qT/kT in halves of T) ----
        qT_r = qT.rearrange("(ck p) t -> p ck t", p=128).bitcast(RDT)
        kT_r = kT.rearrange("(ck p) t -> p ck t", p=128).bitcast(RDT)
        TQ = 512
        for tq in range(T // TQ):
            xh = big.tile([128, CK, TQ], RDT, tag="xT", name="xh")
            nc.sync.dma_start(xh[:], qT_r[:, :, tq * TQ:(tq + 1) * TQ])
            for m in range(2):
                p = ps.tile([128, 512], F32, tag="pscore", name="pproj")
                for ck in range(CK):
                    nc.tensor.matmul(
                        p[:], wq_t[:, ck, m * 128:(m + 1) * 128],
                        xh[:, ck, :], start=(ck == 0), stop=(ck == CK - 1))
                nc.vector.tensor_scalar_add(
                    qpT_t[m][:, tq * TQ:(tq + 1) * TQ], p[:], bq_t[:, m, :])
        for tq in range(T // TQ):
            xh = big.tile([128, CK, TQ], RDT, tag="xT", name="xh")
            nc.sync.dma_start(xh[:], kT_r[:, :, tq * TQ:(tq + 1) * TQ])
            for m in range(2):
                p = ps.tile([128, 512], F32, tag="pscore", name="pproj")
                for ck in range(CK):
                    nc.tensor.matmul(
                        p[:], wk_t[:, ck, m * 128:(m + 1) * 128],
                        xh[:, ck, :], start=(ck == 0), stop=(ck == CK - 1))
                nc.vector.tensor_scalar_add(
                    kpT_t[m][:, tq * TQ:(tq + 1) * TQ], p[:], bk_t[:, m, :])
            # vp for this quarter: vp[t,:] = k @ Wkv + bkv (K=1 ones trick)
            for ti in range(tq * 4, (tq + 1) * 4):
                tl = ti * 128 - tq * TQ
                p = ps.tile([128, 512], F32, tag="pscore", name="pproj")
                for ck in range(CK):
                    nc.tensor.matmul(
                        p[:, :HD], xh[:, ck, tl:tl + 128], wkv_t[:, ck, :],
                        start=(ck == 0), stop=False)
                nc.tensor.matmul(p[:, :HD], ones_t[:], bkv_t[:],
                                 start=False, stop=True)
                nc.vector.tensor_copy(
                    vp_t[:, ti, :, 0:D],
                    p[:, :HD].rearrange("p (h d) -> p h d", h=HPC))

        # ---- Phase 2: attention per head pair, q-windows of 512 ----
        for hp in range(2):
            for j in range(4):
                # A-pass: per q-tile, S rows -> exp -> normalize -> DMA
                for i in range(4 * j, 4 * j + 4):
                    win = (i + 1) * 128
                    nch = (win + 511) // 512
                    dk = (i * 128) // 512
                    strips = {}
                    rs = {}
                    for g in range(2):
                        strips[g] = apool.tile([128, T], F32, tag="astrip",
                                               name="astrip")
                        rs[g] = small.tile([128, 4], F32, tag=f"rs{g}",
                                           name=f"rs{g}")
                    for g in range(2):
                        off = 64 * g
                        for kc in range(nch):
                            n = min(512, win - kc * 512)
                            p = ps.tile([128, 512], F32, tag="pscore",
                                        name="pscore")
                            nc.tensor.matmul(
                                p[:, :n],
                                qpT_t[hp][off:off + 64, i * 128:(i + 1) * 128],
                                kpT_t[hp][off:off + 64, kc * 512:kc * 512 + n],
                                start=True, stop=True)
                            if kc == dk:
                                pos = i * 128 - dk * 512
                                nc.vector.tensor_tensor(
                                    out=p[:, pos:pos + 128],
                                    in0=p[:, pos:pos + 128],
                                    in1=diag_t[:], op=mybir.AluOpType.add)
                            nc.scalar.activation(
                                strips[g][:, kc * 512:kc * 512 + n],
                                p[:, :n], ACT.Exp,
                                accum_out=rs[g][:, kc:kc + 1])
                    # row sums via reduce over the causal window
                    for g in range(2):
                        h = 2 * hp + g
                        rtot = small.tile([128, 1], F32, tag="rtot", name="rtot")
                        nc.vector.tensor_reduce(rtot[:], rs[g][:, :nch],
                                                axis=mybir.AxisListType.X,
                                                op=mybir.AluOpType.add)
                        rcp = small.tile([128, 1], F32, tag="rcp", name="rcp")
                        nc.vector.reciprocal(rcp[:], rtot[:])
                        nc.vector.tensor_scalar_mul(strips[g][:, :win],
                                                    strips[g][:, :win], rcp[:])
                        nc.sync.dma_start(
                            attnw[h, i * 128:(i + 1) * 128, 0:win],
                            strips[g][:, :win])
                # ST-pass + PV: S^T tiles -> exp -> E^T, PV accumulates ctxT
                pcv = {}
                for g in range(2):
                    pcv[g] = psc.tile([D + 1, 512], F32, tag=f"pcv{g}",
                                      name=f"pcv{g}")
                nkc = 4 * j + 4
                for kc in range(nkc):
                    ets = {}
                    pts = {}
                    for g in range(2):
                        off = 64 * g
                        p = ps.tile([128, 512], F32, tag="pst", name="pst", bufs=3)
                        pts[g] = p
                        lo = max(0, kc - 4 * j) * 128
                        nc.tensor.matmul(
                            p[:, lo:],
                            kpT_t[hp][off:off + 64, kc * 128:(kc + 1) * 128],
                            qpT_t[hp][off:off + 64, j * 512 + lo:(j + 1) * 512],
                            start=True, stop=True)
                    ld = kc - 4 * j   # local diagonal block index
                    for g in range(2):
                        p = pts[g]
                        et = etp.tile([128, 512], RDT, tag="et", name="et")
                        ets[g] = et
                        if ld >= 0:
                            nc.vector.tensor_tensor(
                                out=p[:, ld * 128:(ld + 1) * 128],
                                in0=p[:, ld * 128:(ld + 1) * 128],
                                in1=diagT_t[:], op=mybir.AluOpType.add)
                            if ld > 0:
                                nc.vector.tensor_copy(et[:, :ld * 128],
                                                      zeror_t[:, :ld * 128])
                            nc.scalar.activation(et[:, ld * 128:],
                                                 p[:, ld * 128:], ACT.Exp)
                        else:
                            nc.scalar.activation(et[:], p[:], ACT.Exp)
                    for g in range(2):
                        h = 2 * hp + g
                        nc.tensor.matmul(pcv[g][:], vp_t[:, kc, h, :], ets[g][:],
                                         start=(kc == 0), stop=(kc == nkc - 1))
                for g in range(2):
                    h = 2 * hp + g
                    off = 64 * g
                    # reciprocal of the denominator row, broadcast on GpSimd
                    rT = small.tile([1, 512], F32, tag="rT", name="rT", bufs=2)
                    nc.vector.reciprocal(rT[:], pcv[g][D:D + 1, :])
                    pb_sb = small.tile([D, 512], F32, tag="pbsb", name="pb_sb", bufs=2)
                    nc.gpsimd.partition_broadcast(pb_sb[:], rT[:])
                    nc.vector.tensor_tensor(
                        out=ctxT_t[hp][off:off + 64, j * 512:(j + 1) * 512],
                        in0=pcv[g][0:D, :], in1=pb_sb[:],
                        op=mybir.AluOpType.mult)

        # ---- Phase 3: output projection (partial; host sums across cores) ----
        for i in range(QT):
            o_sb = opool.tile([128, C], F32, tag="osb", name="o_sb")
            for n2 in range(2):
                p = ps.tile([128, 512], F32, tag="pscore", name="pout")
                for dc in range(2):
                    nc.tensor.matmul(p[:],
                                     ctxT_t[dc][:, i * 128:(i + 1) * 128],
                                     wo_t[:, dc, n2 * 512:(n2 + 1) * 512],
                                     start=(dc == 0), stop=(dc == 1))
                nc.vector.tensor_copy(o_sb[:, n2 * 512:(n2 + 1) * 512], p[:])
            nc.sync.dma_start(outp[i * 128:(i + 1) * 128, :], o_sb[:])

    nc.compile()
    return nc


_cached = {}


def _get_prog():
    if "nc" not in _cached:
        _cached["nc"] = _build()
    return _cached["nc"]


def _prep_inputs(q, k, Wq, bq, Wk, bk, Wv, bv, Wo, bo, mask):
    """Build the 8 per-core input maps (host-side sharding)."""
    q = np.asarray(q, np.float32)
    k = np.asarray(k, np.float32)
    Wq = np.asarray(Wq, np.float64)
    Wk = np.asarray(Wk, np.float64)
    Wv = np.asarray(Wv, np.float64)
    Wo = np.asarray(Wo, np.float64)
    bq = np.asarray(bq, np.float64)
    bk = np.asarray(bk, np.float64)
    bv = np.asarray(bv, np.float64)
    scale = 1.0 / math.sqrt(D)
    in_maps = []
    for core in range(NCORES):
        b = core // 4
        h0 = HPC * (core % 4)
        sl = slice(h0 * D, (h0 + HPC) * D)
        in_maps.append({
            "qT": np.ascontiguousarray(q[b].T),
            "kT": np.ascontiguousarray(k[b].T),
            "wq": np.ascontiguousarray(Wq[:, sl] * scale).astype(np.float32),
            "wk": np.ascontiguousarray(Wk[:, sl]).astype(np.float32),
            "wkv": np.ascontiguousarray(Wk @ Wv[:, sl]).astype(np.float32),
            "wo": np.ascontiguousarray(Wo[sl, :]).astype(np.float32),
            "bq": (bq[sl] * scale).astype(np.float32).reshape(2, 128, 1),
            "bk": bk[sl].astype(np.float32).reshape(2, 128, 1),
            "bkv": (bk @ Wv[:, sl] + bv[sl]).astype(np.float32).reshape(1, HD),
        })
    return in_maps


def _run(inputs, trace=False, trace_kwargs=None):
    nc = _get_prog()
    in_maps = _prep_inputs(**inputs)
    res = run_bass_kernel_spmd(nc, in_maps, list(range(NCORES)), trace=trace,
                               **(trace_kwargs or {}))
    bo = np.asarray(inputs["bo"], np.float64)
    out = np.empty((B, T, C), np.float32)
    attnw = np.zeros((B, H, T, T), np.float32)
    for b in range(B):
        acc = np.zeros((T, C), np.float64)
        for cc in range(4):
            core = b * 4 + cc
            acc += res.results[core]["outp"]
            dev = res.results[core]["attnw"]
            dst = attnw[b, cc * HPC:(cc + 1) * HPC]
            # device wrote only the causal part; upper triangle stays zero
            for i in range(QT):
                win = (i + 1) * 128
                dst[:, i * 128:(i + 1) * 128, :win] = \
                    dev[:, i * 128:(i + 1) * 128, :win]
        out[b] = (acc + bo).astype(np.float32)
    return (out, attnw), res


def kernel(**inputs):
    (out, attnw), _ = _run(inputs, trace=False)
    return out, attnw
